# revision 7
# baseline (speedup 1.0000x reference)
"""Trainium2 Bass kernel for nn_EnergyFunction (dense transformer block).

Reference math (B=2, S=2048, D=1024, H=8 heads, hd=128):
    K  = x @ Wk.T            [B,S,D] -> heads [B,H,S,hd]
    V  = x @ Wv.T
    E  = (K K^T)/sqrt(hd)    per head, causal mask (q >= k allowed)
    P  = softmax(-E, axis=k)
    O  = P @ V               -> [B,S,D]
    out = (O + x @ Wself.T) @ Wout.T

Sharding (8 cores): core c -> batch b=c//4, head pair hp=c%4 (heads 2hp,2hp+1,
dims ds=[256*hp, 256*hp+256)).  Each core computes
    partial_c = (O_heads + x @ Wself.T[:,ds]) @ Wout.T[ds,:]   [S, D]
and the host sums the 4 partials per batch (row/column-parallel Wout split).

v1 design notes (vs the fp32r baseline):
  * fp8e4m3 + DoubleRow matmuls (2 rows/cycle) for every contraction >= 256:
    K/V projections (x8/wk8/wv8 quantized on host), P@V and the softmax
    denominator (ones) matmuls (P stored fp8 pairs [P,2,QC]).  Scores stay
    fp8 non-DR (contraction = hd = 128, no DR win).  The Wself path is
    precision-critical (it dominates the output) and stays bf16; U@Wout
    stays fp32r.  Simulated end-to-end rel err ~6e-3 (gate 2e-2).
  * Causal mask applied as +100 additive mask on the PSUM scores (DVE STT)
    BEFORE exp, so exp can write fp8 directly.  Max unnormalized P on this
    data ~46 vs the 240 fp8 limit; an epsilon on the denominator guards the
    q=0 row (single softmax term) against fp8 underflow -> 0*inf NaN.
  * Phases pipelined per q-chunk j: B_j (K proj) -> C_j (V) -> F_{j-1}
    (out proj) -> D_j (attention, software-pipelined units with E_j (self)
    matmuls injected between units as PE filler while ACT runs exp).
    E writes ut first; D's normalized O is added into ut at the epilogue.
  * PSUM budget exactly 8 banks: ps_a 3 (scores/B/C/F rotate), ps_e 1 (E),
    ps_ot 2 (P@V accum), ps_l 2 (denominator accum).
"""

import os
import sys

import numpy as np

if "/opt/trn_rl_repo" not in sys.path:
    sys.path.insert(0, "/opt/trn_rl_repo")

import concourse.bass as bass
import concourse.mybir as mybir
import concourse.tile as tile
from concourse.bass import ts
from concourse.bass_utils import run_bass_kernel_spmd

B, S, D = 2, 2048, 1024
H = 8
HD = D // H          # 128 head dim
HPC = 2              # heads per core
DS = HPC * HD        # 256 dims per core
N_CORES = 8
P = 128              # partitions
QC = 512             # q chunk width
NQC = S // QC        # 4 q chunks
NKT = S // P         # 16 k tiles
NDC = D // P         # 8 contraction chunks over D
NDP = NDC // 2       # 4 DoubleRow contraction pair-chunks

F32 = mybir.dt.float32
F32R = mybir.dt.float32r
BF16 = mybir.dt.bfloat16
FP8 = mybir.dt.float8e4
EXP = mybir.ActivationFunctionType.Exp
DR = mybir.MatmulPerfMode.DoubleRow
MUL = mybir.AluOpType.mult
ADD = mybir.AluOpType.add


def _legalize_waits(nc):
    """This toolchain's walrus rejects >1 semaphore wait on several
    instruction structs (Drain/CTRL allows none, Matmult/Ldweights S3_LW
    allows one).  Hoist excess waits onto same-engine NOPs placed
    immediately before the offending instruction."""
    for blk in nc.main_func.blocks:
        insts = blk.instructions
        new = []
        changed = False
        for ins in insts:
            si = ins.sync_info
            if si is not None and si.on_wait:
                allow = 0 if ins.opcode == "Drain" else 1
                waits = list(si.on_wait)
                if len(waits) > allow:
                    cut = len(waits) - allow
                    for k, w in enumerate(waits[:cut]):
                        nop = mybir.InstNoOp(
                            name=f"{ins.name}-wsplit{k}", engine=ins.engine
                        )
                        nop.sync_info = mybir.SyncInfo(on_wait=[w], on_update=[])
                        new.append(nop)
                    ins.sync_info = mybir.SyncInfo(
                        on_wait=waits[cut:], on_update=list(si.on_update)
                    )
                    changed = True
            new.append(ins)
        if changed:
            blk.instructions = new


def _build(repeats=1, loop_n=None, d_mode=None, **_knobs):
    """loop_n: timing-only mode — wrap the body in a device-side For_i loop
    so NEFF execution time dominates the ~200 ms axon RPC floor."""
    nc = bass.Bass()

    x8 = nc.dram_tensor("x8", [D, S], FP8, kind="ExternalInput")
    xb = nc.dram_tensor("xb", [D, S], BF16, kind="ExternalInput")
    wk8 = nc.dram_tensor("wk8", [D, DS], FP8, kind="ExternalInput")
    wv8 = nc.dram_tensor("wv8", [D, DS], FP8, kind="ExternalInput")
    wsb = nc.dram_tensor("wsb", [D, DS], BF16, kind="ExternalInput")
    woutT = nc.dram_tensor("woutT", [DS, D], F32R, kind="ExternalInput")
    ones8 = nc.dram_tensor("ones8", [P, 2 * P], FP8, kind="ExternalInput")
    mtri = nc.dram_tensor("mtri", [P, P], BF16, kind="ExternalInput")
    modd = nc.dram_tensor("modd", [P, 2 * P], BF16, kind="ExternalInput")
    part = nc.dram_tensor("part", [S, D], BF16, kind="ExternalOutput")
    # tiny completion-marker output for timing harnesses
    tick = nc.dram_tensor("tick", [1, 8], BF16, kind="ExternalOutput")

    with tile.TileContext(nc) as tc:
        with (
            tc.tile_pool(name="persist", bufs=1) as pp,
            tc.tile_pool(name="pt_pool", bufs=6) as pt_pool,
            tc.tile_pool(name="rb_pool", bufs=4) as rb_pool,
            tc.tile_pool(name="s_pool", bufs=2) as s_pool,
            tc.tile_pool(name="out_pool", bufs=3) as out_pool,
            tc.tile_pool(name="ps_a", bufs=3, space="PSUM") as ps_a,
            tc.tile_pool(name="ps_e", bufs=1, space="PSUM") as ps_e,
            tc.tile_pool(name="ps_ot", bufs=2, space="PSUM") as ps_ot,
            tc.tile_pool(name="ps_l", bufs=2, space="PSUM") as ps_l,
        ):
            # ---- persistent SBUF tensors ----
            x8_sb = pp.tile([P, NDC, S], FP8, name="x8_sb")
            xb_sb = pp.tile([P, NDC, S], BF16, name="xb_sb")
            wk8_sb = pp.tile([P, NDC, DS], FP8, name="wk8_sb")
            wv8_sb = pp.tile([P, NDC, DS], FP8, name="wv8_sb")
            wsb_sb = pp.tile([P, NDC, DS], BF16, name="wsb_sb")
            wout_sb = pp.tile([P, HPC, D], F32R, name="wout_sb")
            kt8_sb = pp.tile([P, HPC, S], FP8, name="kt8_sb")
            v8_sb = pp.tile([P, NKT, DS], FP8, name="v8_sb")
            ut_sb = pp.tile([P, HPC, S], F32R, name="ut_sb")
            ones8_sb = pp.tile([P, 2, P], FP8, name="ones8_sb")
            mtri_sb = pp.tile([P, P], BF16, name="mtri_sb")
            modd_sb = pp.tile([P, 2 * P], BF16, name="modd_sb")

            def pcopy(dst, src_):
                # psum->sbuf copies: split halves across ACT+DVE so the
                # PSUM bank frees in half the latency
                w = dst.shape[-1]
                h = w // 2
                nc.scalar.copy(dst[:, :h], src_[:, :h])
                nc.vector.tensor_copy(dst[:, h:], src_[:, h:])

            import contextlib

            loop_ctx = (
                tc.For_i(0, loop_n, 1) if loop_n else contextlib.nullcontext()
            )
            with loop_ctx:
              for _rep in range(repeats):
                # ---- weights + consts (first-use order) ----
                for c in range(NDC):
                    nc.sync.dma_start(wk8_sb[:, c, :], wk8[ts(c, P), :])
                for c in range(NDC):
                    nc.sync.dma_start(wv8_sb[:, c, :], wv8[ts(c, P), :])
                nc.sync.dma_start(ones8_sb[:], ones8[:])
                nc.sync.dma_start(mtri_sb[:], mtri[:])
                nc.sync.dma_start(modd_sb[:], modd[:])
                for c in range(NDC):
                    nc.sync.dma_start(wsb_sb[:, c, :], wsb[ts(c, P), :])
                for c in range(HPC):
                    nc.sync.dma_start(wout_sb[:, c, :], woutT[ts(c, P), :])

                for j in range(NQC):
                    # ---- x chunk DMA ----
                    for c in range(NDC):
                        nc.sync.dma_start(
                            x8_sb[:, c, ts(j, QC)], x8[ts(c, P), ts(j, QC)]
                        )
                    for c in range(NDC):
                        nc.sync.dma_start(
                            xb_sb[:, c, ts(j, QC)], xb[ts(c, P), ts(j, QC)]
                        )

                    # ---- B_j: KT[h] chunk j = (Wk_h*hd^-.25) @ x.T  (fp8 DR)
                    for h in range(HPC):
                        ps = ps_a.tile([P, QC], F32, name="ps_b", tag="ps_a")
                        for cp in range(NDP):
                            nc.tensor.matmul(
                                ps[:],
                                wk8_sb[:, 2 * cp : 2 * cp + 2, ts(h, HD)],
                                x8_sb[:, 2 * cp : 2 * cp + 2, ts(j, QC)],
                                start=(cp == 0),
                                stop=(cp == NDP - 1),
                                perf_mode=DR,
                            )
                        pcopy(kt8_sb[:, h, ts(j, QC)], ps[:])

                    # ---- C_j: V tiles 4j..4j+3  (fp8 DR) ----
                    for st in range(4 * j, 4 * j + 4):
                        ps = ps_a.tile([P, QC], F32, name="ps_c", tag="ps_a")
                        for cp in range(NDP):
                            nc.tensor.matmul(
                                ps[:, :DS],
                                x8_sb[:, 2 * cp : 2 * cp + 2, ts(st, P)],
                                wv8_sb[:, 2 * cp : 2 * cp + 2, :],
                                start=(cp == 0),
                                stop=(cp == NDP - 1),
                                perf_mode=DR,
                            )
                        pcopy(v8_sb[:, st, :], ps[:, :DS])

                    # ---- F_{j-1}: partial rows for the previous chunk ----
                    def emit_F(jj):
                        for qt in range(4 * jj, 4 * jj + 4):
                            ob = out_pool.tile([P, D], BF16, name="ob", tag="ob")
                            for nch in range(2):
                                ps = ps_a.tile(
                                    [P, QC], F32, name="ps_f", tag="ps_a"
                                )
                                for m in range(HPC):
                                    nc.tensor.matmul(
                                        ps[:],
                                        ut_sb[:, m, ts(qt, P)],
                                        wout_sb[:, m, ts(nch, QC)],
                                        start=(m == 0),
                                        stop=(m == HPC - 1),
                                    )
                                pcopy(ob[:, ts(nch, QC)], ps[:])
                            nc.sync.dma_start(part[ts(qt, P), :], ob[:])
                            if qt == NKT - 1:
                                nc.sync.dma_start(tick[:, :], ob[0:1, 0:8])

                    if j > 0:
                        emit_F(j - 1)

                    # ---- D_j: attention (software-pipelined units) with
                    #      E_j (self-proj) matmuls injected as PE filler ----
                    npairs = 2 * j + 2
                    ot = {
                        h: ps_ot.tile([P, QC], F32, name=f"ot{h}", tag="ps_ot")
                        for h in range(HPC)
                    }
                    lb = {
                        h: ps_l.tile([P, QC], F32, name=f"lb{h}", tag="ps_l")
                        for h in range(HPC)
                    }

                    def emit_E_head(h):
                        ps = ps_e.tile([P, QC], F32, name="ps_ej", tag="ps_e")
                        for c in range(NDC):
                            nc.tensor.matmul(
                                ps[:],
                                wsb_sb[:, c, ts(h, HD)],
                                xb_sb[:, c, ts(j, QC)],
                                start=(c == 0),
                                stop=(c == NDC - 1),
                            )
                        pcopy(ut_sb[:, h, ts(j, QC)], ps[:])

                    def emit_pvl(p, h, c00, pt2):
                        kt0 = 2 * p
                        nc.tensor.matmul(
                            ot[h][:, c00:],
                            v8_sb[:, kt0 : kt0 + 2, ts(h, HD)],
                            pt2[:, :, c00:],
                            start=(p == 0),
                            stop=(p == npairs - 1),
                            perf_mode=DR,
                        )
                        nc.tensor.matmul(
                            lb[h][:, c00:],
                            ones8_sb[:],
                            pt2[:, :, c00:],
                            start=(p == 0),
                            stop=(p == npairs - 1),
                            perf_mode=DR,
                        )

                    units = [(p, h) for p in range(npairs) for h in range(HPC)]
                    inj = {len(units) // 3: 0, (2 * len(units)) // 3: 1}
                    pending = None
                    for idx, (p, h) in enumerate(units):
                        kt0 = 2 * p
                        c00 = max(0, P * kt0 - QC * j)
                        diag = kt0 >= 4 * j
                        pt2 = pt_pool.tile(
                            [P, 2, QC], FP8, name="pt2", tag="pt2"
                        )
                        eps = []
                        for sub in range(2):
                            ep = ps_a.tile([P, QC], F32, name="ep", tag="ps_a")
                            nc.tensor.matmul(
                                ep[:, c00:],
                                kt8_sb[:, h, ts(kt0 + sub, P)],
                                kt8_sb[:, h, QC * j + c00 : QC * (j + 1)],
                                start=True,
                                stop=True,
                            )
                            eps.append(ep)
                        if diag:
                            # +100 additive causal mask on PSUM scores
                            nc.vector.scalar_tensor_tensor(
                                eps[0][:, c00 : c00 + P],
                                eps[0][:, c00 : c00 + P],
                                1.0,
                                mtri_sb[:],
                                op0=MUL,
                                op1=ADD,
                            )
                            nc.vector.scalar_tensor_tensor(
                                eps[1][:, c00 : c00 + 2 * P],
                                eps[1][:, c00 : c00 + 2 * P],
                                1.0,
                                modd_sb[:],
                                op0=MUL,
                                op1=ADD,
                            )
                        for sub in range(2):
                            nc.scalar.activation(
                                pt2[:, sub, c00:],
                                eps[sub][:, c00:],
                                EXP,
                                scale=-1.0,
                            )
                        if pending is not None:
                            emit_pvl(*pending)
                        pending = (p, h, c00, pt2)
                        if idx in inj:
                            emit_E_head(inj[idx])
                    emit_pvl(*pending)

                    # ---- epilogue: ut += O/l ----
                    for h in range(HPC):
                        # +eps: q=0 has a single softmax term whose fp8 value
                        # can underflow to 0; avoid 0*inf = NaN in row 0
                        lc = rb_pool.tile([P, QC], F32, name="lc", tag="lc")
                        nc.vector.tensor_scalar_add(lc[:], lb[h][:], 1e-30)
                        li = rb_pool.tile([P, QC], F32, name="li", tag="li")
                        nc.vector.reciprocal(li[:], lc[:])
                        sn = s_pool.tile([P, QC], F32, name="sn", tag="sn")
                        for sub in range(4):
                            nc.vector.tensor_mul(
                                sn[:, ts(sub, P)],
                                ot[h][:, ts(sub, P)],
                                li[:, ts(sub, P)],
                            )
                        for sub in range(4):
                            sl = slice(QC * j + P * sub, QC * j + P * (sub + 1))
                            nc.vector.tensor_add(
                                ut_sb[:, h, sl],
                                ut_sb[:, h, sl].bitcast(F32),
                                sn[:, ts(sub, P)],
                            )

                emit_F(NQC - 1)

    _legalize_waits(nc)
    return nc


_NC = None
D_MODE = "sym"  # compat knob for test.py; single implementation now


def _get_nc():
    global _NC
    if _NC is None:
        _NC = _build()
    return _NC


def build_in_maps(x, Wk, Wv, Wself, Wout):
    import ml_dtypes

    F8NP = ml_dtypes.float8_e4m3
    BFNP = ml_dtypes.bfloat16
    x = np.ascontiguousarray(np.asarray(x, dtype=np.float32))
    Wk = np.asarray(Wk, dtype=np.float32)
    Wv = np.asarray(Wv, dtype=np.float32)
    Wself = np.asarray(Wself, dtype=np.float32)
    Wout = np.asarray(Wout, dtype=np.float32)

    ks = np.float32(HD ** -0.25)
    ones8 = np.ones((P, 2 * P), F8NP)
    triu = np.triu(np.ones((P, P), np.float32))
    mtri = ((1.0 - triu) * 100.0).astype(BFNP)
    modd = np.concatenate(
        [np.full((P, P), 100.0, np.float32), (1.0 - triu) * 100.0], axis=1
    ).astype(BFNP)

    in_maps = []
    for c in range(N_CORES):
        b, hp = divmod(c, 4)
        ds = slice(DS * hp, DS * (hp + 1))
        xT = np.ascontiguousarray(x[b].T)
        in_maps.append(
            {
                "x8": xT.astype(F8NP),
                "xb": xT.astype(BFNP),
                "wk8": np.ascontiguousarray((Wk[ds, :] * ks).T).astype(F8NP),
                "wv8": np.ascontiguousarray(Wv[ds, :].T).astype(F8NP),
                "wsb": np.ascontiguousarray(Wself[ds, :].T).astype(BFNP),
                "woutT": np.ascontiguousarray(Wout[:, ds].T),
                "ones8": ones8,
                "mtri": mtri,
                "modd": modd,
            }
        )
    return in_maps


def kernel(x, Wk, Wv, Wself, Wout):
    nc = _get_nc()
    in_maps = build_in_maps(x, Wk, Wv, Wself, Wout)
    res = run_bass_kernel_spmd(nc, in_maps, core_ids=list(range(N_CORES)))

    out = np.empty((B, S, D), np.float32)
    for b in range(B):
        acc = np.zeros((S, D), np.float32)
        for hp in range(4):
            acc += res.results[4 * b + hp]["part"].astype(np.float32)
        out[b] = acc
    return out


# revision 9
# speedup vs baseline: 1.5029x; 1.5029x over previous
"""Trainium2 Bass kernel for nn_EnergyFunction (dense transformer block).

Reference math (B=2, S=2048, D=1024, H=8 heads, hd=128):
    K  = x @ Wk.T            [B,S,D] -> heads [B,H,S,hd]
    V  = x @ Wv.T
    E  = (K K^T)/sqrt(hd)    per head, causal mask (q >= k allowed)
    P  = softmax(-E, axis=k)
    O  = P @ V               -> [B,S,D]
    out = (O + x @ Wself.T) @ Wout.T

Sharding (8 cores): core c -> batch b=c//4, head pair hp=c%4 (heads 2hp,2hp+1,
dims ds=[256*hp, 256*hp+256)).  Each core computes
    partial_c = (O_heads + x @ Wself.T[:,ds]) @ Wout.T[ds,:]   [S, D]
and the host sums the 4 partials per batch (row/column-parallel Wout split).

v1 design notes (vs the fp32r baseline):
  * fp8e4m3 + DoubleRow matmuls (2 rows/cycle) for every contraction >= 256:
    K/V projections (x8/wk8/wv8 quantized on host), P@V and the softmax
    denominator (ones) matmuls (P stored fp8 pairs [P,2,QC]).  Scores stay
    fp8 non-DR (contraction = hd = 128, no DR win).  The Wself path is
    precision-critical (it dominates the output) and stays bf16; U@Wout
    stays fp32r.  Simulated end-to-end rel err ~6e-3 (gate 2e-2).
  * Causal mask applied as +100 additive mask on the PSUM scores (DVE STT)
    BEFORE exp, so exp can write fp8 directly.  Max unnormalized P on this
    data ~46 vs the 240 fp8 limit; an epsilon on the denominator guards the
    q=0 row (single softmax term) against fp8 underflow -> 0*inf NaN.
  * Phases pipelined per q-chunk j: B_j (K proj) -> C_j (V) -> F_{j-1}
    (out proj) -> D_j (attention, software-pipelined units with E_j (self)
    matmuls injected between units as PE filler while ACT runs exp).
    E writes ut first; D's normalized O is added into ut at the epilogue.
  * PSUM budget exactly 8 banks: ps_a 3 (scores/B/C/F rotate), ps_e 1 (E),
    ps_ot 2 (P@V accum), ps_l 2 (denominator accum).
"""

import os
import sys

import numpy as np

if "/opt/trn_rl_repo" not in sys.path:
    sys.path.insert(0, "/opt/trn_rl_repo")

import concourse.bass as bass
import concourse.mybir as mybir
import concourse.tile as tile
from concourse.bass import ts
from concourse.bass_utils import run_bass_kernel_spmd

B, S, D = 2, 2048, 1024
H = 8
HD = D // H          # 128 head dim
HPC = 2              # heads per core
DS = HPC * HD        # 256 dims per core
N_CORES = 8
P = 128              # partitions
QC = 512             # q chunk width
NQC = S // QC        # 4 q chunks
NKT = S // P         # 16 k tiles
NDC = D // P         # 8 contraction chunks over D
NDP = NDC // 2       # 4 DoubleRow contraction pair-chunks

F32 = mybir.dt.float32
F32R = mybir.dt.float32r
BF16 = mybir.dt.bfloat16
FP8 = mybir.dt.float8e4
EXP = mybir.ActivationFunctionType.Exp
DR = mybir.MatmulPerfMode.DoubleRow
MUL = mybir.AluOpType.mult
ADD = mybir.AluOpType.add


def _legalize_waits(nc):
    """This toolchain's walrus rejects >1 semaphore wait on several
    instruction structs (Drain/CTRL allows none, Matmult/Ldweights S3_LW
    allows one).  Hoist excess waits onto same-engine NOPs placed
    immediately before the offending instruction."""
    for blk in nc.main_func.blocks:
        insts = blk.instructions
        new = []
        changed = False
        for ins in insts:
            si = ins.sync_info
            if si is not None and si.on_wait:
                allow = 0 if ins.opcode == "Drain" else 1
                waits = list(si.on_wait)
                if len(waits) > allow:
                    cut = len(waits) - allow
                    for k, w in enumerate(waits[:cut]):
                        nop = mybir.InstNoOp(
                            name=f"{ins.name}-wsplit{k}", engine=ins.engine
                        )
                        nop.sync_info = mybir.SyncInfo(on_wait=[w], on_update=[])
                        new.append(nop)
                    ins.sync_info = mybir.SyncInfo(
                        on_wait=waits[cut:], on_update=list(si.on_update)
                    )
                    changed = True
            new.append(ins)
        if changed:
            blk.instructions = new


def _build(repeats=1, loop_n=None, d_mode=None, phases="BCDEF", **_knobs):
    """loop_n: timing-only mode — wrap the body in a device-side For_i loop
    so NEFF execution time dominates the ~200 ms axon RPC floor."""
    nc = bass.Bass()

    x8 = nc.dram_tensor("x8", [D, S], FP8, kind="ExternalInput")
    xb = nc.dram_tensor("xb", [D, S], BF16, kind="ExternalInput")
    wk8 = nc.dram_tensor("wk8", [D, DS], FP8, kind="ExternalInput")
    wv8 = nc.dram_tensor("wv8", [D, DS], FP8, kind="ExternalInput")
    wsb = nc.dram_tensor("wsb", [D, DS], BF16, kind="ExternalInput")
    woutT = nc.dram_tensor("woutT", [DS, D], F32R, kind="ExternalInput")
    ones8 = nc.dram_tensor("ones8", [P, 2 * P], FP8, kind="ExternalInput")
    mtri = nc.dram_tensor("mtri", [P, P], BF16, kind="ExternalInput")
    m00 = nc.dram_tensor("m00", [P, P], BF16, kind="ExternalInput")
    modd = nc.dram_tensor("modd", [P, 2 * P], BF16, kind="ExternalInput")
    part = nc.dram_tensor("part", [S, D], BF16, kind="ExternalOutput")
    # tiny completion-marker output for timing harnesses
    tick = nc.dram_tensor("tick", [1, 8], BF16, kind="ExternalOutput")

    with tile.TileContext(nc) as tc:
        with (
            tc.tile_pool(name="persist", bufs=1) as pp,
            tc.tile_pool(name="pt_pool", bufs=6) as pt_pool,
            tc.tile_pool(name="rb_pool", bufs=4) as rb_pool,
            tc.tile_pool(name="s_pool", bufs=2) as s_pool,
            tc.tile_pool(name="out_pool", bufs=3) as out_pool,
            tc.tile_pool(name="ps_a", bufs=3, space="PSUM") as ps_a,
            tc.tile_pool(name="ps_e", bufs=1, space="PSUM") as ps_e,
            tc.tile_pool(name="ps_ot", bufs=2, space="PSUM") as ps_ot,
            tc.tile_pool(name="ps_l", bufs=2, space="PSUM") as ps_l,
        ):
            # ---- persistent SBUF tensors ----
            x8_sb = pp.tile([P, NDC, S], FP8, name="x8_sb")
            xb_sb = pp.tile([P, NDC, S], BF16, name="xb_sb")
            wk8_sb = pp.tile([P, NDC, DS], FP8, name="wk8_sb")
            wv8_sb = pp.tile([P, NDC, DS], FP8, name="wv8_sb")
            wsb_sb = pp.tile([P, NDC, DS], BF16, name="wsb_sb")
            wout_sb = pp.tile([P, HPC, D], F32R, name="wout_sb")
            kt8_sb = pp.tile([P, HPC, S], FP8, name="kt8_sb")
            v8_sb = pp.tile([P, NKT, DS], FP8, name="v8_sb")
            ut_sb = pp.tile([P, HPC, S], F32R, name="ut_sb")
            ones8_sb = pp.tile([P, 2, P], FP8, name="ones8_sb")
            mtri_sb = pp.tile([P, P], BF16, name="mtri_sb")
            m00_sb = pp.tile([P, P], BF16, name="m00_sb")
            modd_sb = pp.tile([P, 2 * P], BF16, name="modd_sb")

            def pcopy(dst, src_):
                # psum->sbuf copies: split halves across ACT+DVE so the
                # PSUM bank frees in half the latency
                w = dst.shape[-1]
                h = w // 2
                nc.scalar.copy(dst[:, :h], src_[:, :h])
                nc.vector.tensor_copy(dst[:, h:], src_[:, h:])

            import contextlib

            loop_ctx = (
                tc.For_i(0, loop_n, 1) if loop_n else contextlib.nullcontext()
            )
            with loop_ctx:
              for _rep in range(repeats):
                # ---- weights + consts (first-use order) ----
                for c in range(NDC):
                    nc.sync.dma_start(wk8_sb[:, c, :], wk8[ts(c, P), :])
                for c in range(NDC):
                    nc.sync.dma_start(wv8_sb[:, c, :], wv8[ts(c, P), :])
                nc.sync.dma_start(ones8_sb[:], ones8[:])
                nc.sync.dma_start(mtri_sb[:], mtri[:])
                nc.sync.dma_start(m00_sb[:], m00[:])
                nc.sync.dma_start(modd_sb[:], modd[:])
                for c in range(NDC):
                    nc.sync.dma_start(wsb_sb[:, c, :], wsb[ts(c, P), :])
                for c in range(HPC):
                    nc.sync.dma_start(wout_sb[:, c, :], woutT[ts(c, P), :])

                for j in range(NQC):
                    # ---- x chunk DMA ----
                    for c in range(NDC):
                        nc.sync.dma_start(
                            x8_sb[:, c, ts(j, QC)], x8[ts(c, P), ts(j, QC)]
                        )
                    for c in range(NDC):
                        nc.sync.dma_start(
                            xb_sb[:, c, ts(j, QC)], xb[ts(c, P), ts(j, QC)]
                        )

                    # ---- B_j: KT[h] chunk j = (Wk_h*hd^-.25) @ x.T  (fp8 DR)
                    for h in range(HPC if "B" in phases else 0):
                        ps = ps_a.tile([P, QC], F32, name="ps_b", tag="ps_a")
                        for cp in range(NDP):
                            nc.tensor.matmul(
                                ps[:],
                                wk8_sb[:, 2 * cp : 2 * cp + 2, ts(h, HD)],
                                x8_sb[:, 2 * cp : 2 * cp + 2, ts(j, QC)],
                                start=(cp == 0),
                                stop=(cp == NDP - 1),
                                perf_mode=DR,
                            )
                        pcopy(kt8_sb[:, h, ts(j, QC)], ps[:])

                    # ---- C_j: V tiles 4j..4j+3  (fp8 DR) ----
                    for st in range(*((4 * j, 4 * j + 4) if "C" in phases else (0, 0))):
                        ps = ps_a.tile([P, QC], F32, name="ps_c", tag="ps_a")
                        for cp in range(NDP):
                            nc.tensor.matmul(
                                ps[:, :DS],
                                x8_sb[:, 2 * cp : 2 * cp + 2, ts(st, P)],
                                wv8_sb[:, 2 * cp : 2 * cp + 2, :],
                                start=(cp == 0),
                                stop=(cp == NDP - 1),
                                perf_mode=DR,
                            )
                        pcopy(v8_sb[:, st, :], ps[:, :DS])

                    # ---- F_{j-1}: partial rows for the previous chunk ----
                    def emit_F(jj):
                        for qt in range(4 * jj, 4 * jj + 4):
                            ob = out_pool.tile([P, D], BF16, name="ob", tag="ob")
                            for nch in range(2):
                                ps = ps_a.tile(
                                    [P, QC], F32, name="ps_f", tag="ps_a"
                                )
                                for m in range(HPC):
                                    nc.tensor.matmul(
                                        ps[:],
                                        ut_sb[:, m, ts(qt, P)],
                                        wout_sb[:, m, ts(nch, QC)],
                                        start=(m == 0),
                                        stop=(m == HPC - 1),
                                    )
                                pcopy(ob[:, ts(nch, QC)], ps[:])
                            nc.sync.dma_start(part[ts(qt, P), :], ob[:])
                            if qt == NKT - 1:
                                nc.sync.dma_start(tick[:, :], ob[0:1, 0:8])

                    if j > 0 and "F" in phases:
                        emit_F(j - 1)

                    # ---- D_j: attention (software-pipelined units) with
                    #      E_j (self-proj) matmuls injected as PE filler ----
                    npairs = 2 * j + 2
                    ot = {
                        h: ps_ot.tile([P, QC], F32, name=f"ot{h}", tag="ps_ot")
                        for h in range(HPC)
                    }
                    lb = {
                        h: ps_l.tile([P, QC], F32, name=f"lb{h}", tag="ps_l")
                        for h in range(HPC)
                    }

                    def emit_E_head(h):
                        ps = ps_e.tile([P, QC], F32, name="ps_ej", tag="ps_e")
                        for c in range(NDC):
                            nc.tensor.matmul(
                                ps[:],
                                wsb_sb[:, c, ts(h, HD)],
                                xb_sb[:, c, ts(j, QC)],
                                start=(c == 0),
                                stop=(c == NDC - 1),
                            )
                        pcopy(ut_sb[:, h, ts(j, QC)], ps[:])

                    def emit_pvl(p, h, c00, pt2):
                        kt0 = 2 * p
                        nc.tensor.matmul(
                            ot[h][:, c00:],
                            v8_sb[:, kt0 : kt0 + 2, ts(h, HD)],
                            pt2[:, :, c00:],
                            start=(p == 0),
                            stop=(p == npairs - 1),
                            perf_mode=DR,
                        )
                        nc.tensor.matmul(
                            lb[h][:, c00:],
                            ones8_sb[:],
                            pt2[:, :, c00:],
                            start=(p == 0),
                            stop=(p == npairs - 1),
                            perf_mode=DR,
                        )

                    units = [(p, h) for p in range(npairs) for h in range(HPC)]
                    if "D" not in phases:
                        units = []
                    inj = (
                        {len(units) // 3: 0, (2 * len(units)) // 3: 1}
                        if ("E" in phases and units)
                        else ({0: 0, 1: 1} if "E" in phases else {})
                    )
                    pending = None
                    for idx, (p, h) in enumerate(units):
                        kt0 = 2 * p
                        c00 = max(0, P * kt0 - QC * j)
                        diag = kt0 >= 4 * j
                        pt2 = pt_pool.tile(
                            [P, 2, QC], FP8, name="pt2", tag="pt2"
                        )
                        eps = []
                        for sub in range(2):
                            ep = ps_a.tile([P, QC], F32, name="ep", tag="ps_a")
                            nc.tensor.matmul(
                                ep[:, c00:],
                                kt8_sb[:, h, ts(kt0 + sub, P)],
                                kt8_sb[:, h, QC * j + c00 : QC * (j + 1)],
                                start=True,
                                stop=True,
                            )
                            eps.append(ep)
                        if diag:
                            # +100 additive causal mask on PSUM scores.
                            # Tile (0,0) uses m00 (mtri with -4 at [0,0]):
                            # q=0's single softmax term gets scaled by e^4 in
                            # both numerator and denominator (cancels), which
                            # keeps its fp8 value away from underflow->NaN.
                            nc.vector.scalar_tensor_tensor(
                                eps[0][:, c00 : c00 + P],
                                eps[0][:, c00 : c00 + P],
                                1.0,
                                m00_sb[:] if (j == 0 and p == 0) else mtri_sb[:],
                                op0=MUL,
                                op1=ADD,
                            )
                            nc.vector.scalar_tensor_tensor(
                                eps[1][:, c00 : c00 + 2 * P],
                                eps[1][:, c00 : c00 + 2 * P],
                                1.0,
                                modd_sb[:],
                                op0=MUL,
                                op1=ADD,
                            )
                        for sub in range(2):
                            nc.scalar.activation(
                                pt2[:, sub, c00:],
                                eps[sub][:, c00:],
                                EXP,
                                scale=-1.0,
                            )
                        if pending is not None:
                            emit_pvl(*pending)
                        pending = (p, h, c00, pt2)
                        if idx in inj:
                            emit_E_head(inj[idx])
                    if pending is not None:
                        emit_pvl(*pending)
                    if not units:
                        for h in inj.values():
                            emit_E_head(h)

                    # ---- epilogue: ut += O/l ----
                    for h in range(HPC if "D" in phases else 0):
                        # +eps: q=0 has a single softmax term whose fp8 value
                        # can underflow to 0; avoid 0*inf = NaN in row 0
                        lc = rb_pool.tile([P, QC], F32, name="lc", tag="lc")
                        nc.vector.tensor_scalar_add(lc[:], lb[h][:], 1e-30)
                        li = rb_pool.tile([P, QC], F32, name="li", tag="li")
                        nc.vector.reciprocal(li[:], lc[:])
                        sn = s_pool.tile([P, QC], F32, name="sn", tag="sn")
                        for sub in range(4):
                            nc.vector.tensor_mul(
                                sn[:, ts(sub, P)],
                                ot[h][:, ts(sub, P)],
                                li[:, ts(sub, P)],
                            )
                        for sub in range(4):
                            sl = slice(QC * j + P * sub, QC * j + P * (sub + 1))
                            nc.vector.tensor_add(
                                ut_sb[:, h, sl],
                                ut_sb[:, h, sl].bitcast(F32),
                                sn[:, ts(sub, P)],
                            )

                if "F" in phases:
                    emit_F(NQC - 1)

    _legalize_waits(nc)
    return nc


_NC = None
D_MODE = "sym"  # compat knob for test.py; single implementation now


def _get_nc():
    global _NC
    if _NC is None:
        _NC = _build()
    return _NC


def build_in_maps(x, Wk, Wv, Wself, Wout):
    import ml_dtypes

    F8NP = ml_dtypes.float8_e4m3
    BFNP = ml_dtypes.bfloat16
    x = np.ascontiguousarray(np.asarray(x, dtype=np.float32))
    Wk = np.asarray(Wk, dtype=np.float32)
    Wv = np.asarray(Wv, dtype=np.float32)
    Wself = np.asarray(Wself, dtype=np.float32)
    Wout = np.asarray(Wout, dtype=np.float32)

    ks = np.float32(HD ** -0.25)
    ones8 = np.ones((P, 2 * P), F8NP)
    triu = np.triu(np.ones((P, P), np.float32))
    mtri = ((1.0 - triu) * 100.0).astype(BFNP)
    m00f = (1.0 - triu) * 100.0
    m00f[0, 0] = -4.0
    m00 = m00f.astype(BFNP)
    modd = np.concatenate(
        [np.full((P, P), 100.0, np.float32), (1.0 - triu) * 100.0], axis=1
    ).astype(BFNP)

    in_maps = []
    for c in range(N_CORES):
        b, hp = divmod(c, 4)
        ds = slice(DS * hp, DS * (hp + 1))
        xT = np.ascontiguousarray(x[b].T)
        in_maps.append(
            {
                "x8": xT.astype(F8NP),
                "xb": xT.astype(BFNP),
                "wk8": np.ascontiguousarray((Wk[ds, :] * ks).T).astype(F8NP),
                "wv8": np.ascontiguousarray(Wv[ds, :].T).astype(F8NP),
                "wsb": np.ascontiguousarray(Wself[ds, :].T).astype(BFNP),
                "woutT": np.ascontiguousarray(Wout[:, ds].T),
                "ones8": ones8,
                "mtri": mtri,
                "m00": m00,
                "modd": modd,
            }
        )
    return in_maps


def kernel(x, Wk, Wv, Wself, Wout):
    nc = _get_nc()
    in_maps = build_in_maps(x, Wk, Wv, Wself, Wout)
    res = run_bass_kernel_spmd(nc, in_maps, core_ids=list(range(N_CORES)))

    out = np.empty((B, S, D), np.float32)
    for b in range(B):
        acc = np.zeros((S, D), np.float32)
        for hp in range(4):
            acc += res.results[4 * b + hp]["part"].astype(np.float32)
        out[b] = acc
    return out


# revision 23
# speedup vs baseline: 1.9377x; 1.2893x over previous
"""Trainium2 Bass kernel for nn_EnergyFunction (dense transformer block).

Reference math (B=2, S=2048, D=1024, H=8 heads, hd=128):
    K  = x @ Wk.T            [B,S,D] -> heads [B,H,S,hd]
    V  = x @ Wv.T
    E  = (K K^T)/sqrt(hd)    per head, causal mask (q >= k allowed)
    P  = softmax(-E, axis=k)
    O  = P @ V               -> [B,S,D]
    out = (O + x @ Wself.T) @ Wout.T

Sharding (8 cores): core c -> batch b=c//4, head pair hp=c%4 (heads 2hp,2hp+1,
dims ds=[256*hp, 256*hp+256)).  Each core computes
    partial_c = (O_heads + x @ Wself.T[:,ds]) @ Wout.T[ds,:]   [S, D]
and the host sums the 4 partials per batch (row/column-parallel Wout split).

Design notes (vs the fp32r baseline, ~1.3x faster steady-state):
  * fp8e4m3 + DoubleRow matmuls (2 rows/cycle) for every contraction >= 256:
    K/V projections (x8/wk8/wv8 quantized on host), P@V and the softmax
    denominator (ones) matmuls (P stored fp8 pairs [P,2,QC]).  Scores stay
    fp8 non-DR (contraction = hd = 128, no DR win).  The Wself path is
    precision-critical (it dominates the output) and stays bf16 (fp8 there
    measures 4e-2 rel err); U@Wout stays fp32r.  End-to-end rel err 7.2e-3
    on hardware (gate 2e-2).
  * Causal mask applied as one +100 additive-mask DVE op per diagonal k-tile
    pair on the PSUM scores BEFORE exp, so exp writes fp8 directly.  Max
    unnormalized P on this data ~46 vs the 240 fp8 limit.  Tile (0,0) uses
    m00comb (-4 at [0,0]): q=0's single softmax term is scaled by e^4 in
    numerator and denominator (cancels) so its fp8 value cannot underflow to
    0 (which made row 0 go 0*inf=NaN); a j=0 epsilon on lb col 0 is a
    second guard.
  * Heads run SEQUENTIALLY within a q-chunk so only one ot/lb accumulator
    pair is live; score k-tile pairs go in 2-bank [P,2,QC] PSUM tiles with
    a single exp instruction per pair (halves ACT instruction overhead).
    PSUM budget exactly 8 banks: ps_pair 2x2 (scores), ps_ef 2 (B/C/E/F),
    ps_ot 1, ps_l 1.
  * B_j/C_j (next K/V chunks), E_j (self proj) and F_{j-1} (out proj) are
    emitted as PE filler between D's score pairs: the TensorE only reaches
    its full 2.4 GHz p-state when continuously busy, so it must never idle
    while ACT chews through the exps.
  * DMAs are batched into a handful of large multi-dim transfers (the two
    HWDGE rings have a big per-dma_start fixed cost; this took the
    B+C-only variant from 76us to 35us).  Output partials ship as bf16.
"""

import os
import sys

import numpy as np

if "/opt/trn_rl_repo" not in sys.path:
    sys.path.insert(0, "/opt/trn_rl_repo")

import concourse.bass as bass
import concourse.mybir as mybir
import concourse.tile as tile
from concourse.bass import ts
from concourse.bass_utils import run_bass_kernel_spmd

B, S, D = 2, 2048, 1024
H = 8
HD = D // H          # 128 head dim
HPC = 2              # heads per core
DS = HPC * HD        # 256 dims per core
N_CORES = 8
P = 128              # partitions
QC = 512             # q chunk width
NQC = S // QC        # 4 q chunks
NKT = S // P         # 16 k tiles
NDC = D // P         # 8 contraction chunks over D
NDP = NDC // 2       # 4 DoubleRow contraction pair-chunks

F32 = mybir.dt.float32
F32R = mybir.dt.float32r
BF16 = mybir.dt.bfloat16
FP8 = mybir.dt.float8e4
EXP = mybir.ActivationFunctionType.Exp
DR = mybir.MatmulPerfMode.DoubleRow
MUL = mybir.AluOpType.mult
ADD = mybir.AluOpType.add


def _legalize_waits(nc):
    """This toolchain's walrus rejects >1 semaphore wait on several
    instruction structs (Drain/CTRL allows none, Matmult/Ldweights S3_LW
    allows one).  Hoist excess waits onto same-engine NOPs placed
    immediately before the offending instruction."""
    for blk in nc.main_func.blocks:
        insts = blk.instructions
        new = []
        changed = False
        for ins in insts:
            si = ins.sync_info
            if si is not None and si.on_wait:
                allow = 0 if ins.opcode == "Drain" else 1
                waits = list(si.on_wait)
                if len(waits) > allow:
                    cut = len(waits) - allow
                    for k, w in enumerate(waits[:cut]):
                        nop = mybir.InstNoOp(
                            name=f"{ins.name}-wsplit{k}", engine=ins.engine
                        )
                        nop.sync_info = mybir.SyncInfo(on_wait=[w], on_update=[])
                        new.append(nop)
                    ins.sync_info = mybir.SyncInfo(
                        on_wait=waits[cut:], on_update=list(si.on_update)
                    )
                    changed = True
            new.append(ins)
        if changed:
            blk.instructions = new


def _build(repeats=1, loop_n=None, d_mode=None, phases="BCDEF", **_knobs):
    """loop_n: timing-only mode — wrap the body in a device-side For_i loop
    so NEFF execution time dominates the ~200 ms axon RPC floor."""
    nc = bass.Bass()

    x8 = nc.dram_tensor("x8", [D, S], FP8, kind="ExternalInput")
    xb = nc.dram_tensor("xb", [D, S], BF16, kind="ExternalInput")
    wk8 = nc.dram_tensor("wk8", [D, DS], FP8, kind="ExternalInput")
    wv8 = nc.dram_tensor("wv8", [D, DS], FP8, kind="ExternalInput")
    wsb = nc.dram_tensor("wsb", [D, DS], BF16, kind="ExternalInput")
    woutT = nc.dram_tensor("woutT", [DS, D], F32R, kind="ExternalInput")
    ones8 = nc.dram_tensor("ones8", [P, 2 * P], FP8, kind="ExternalInput")
    mcomb = nc.dram_tensor("mcomb", [P, 4 * P], BF16, kind="ExternalInput")
    m00comb = nc.dram_tensor("m00comb", [P, 4 * P], BF16, kind="ExternalInput")
    part = nc.dram_tensor("part", [S, D], BF16, kind="ExternalOutput")
    # tiny completion-marker output for timing harnesses
    tick = nc.dram_tensor("tick", [1, 8], BF16, kind="ExternalOutput")

    with tile.TileContext(nc) as tc:
        with (
            tc.tile_pool(name="persist", bufs=1) as pp,
            tc.tile_pool(name="pt_pool", bufs=8) as pt_pool,
            tc.tile_pool(name="rb_pool", bufs=2) as rb_pool,
            tc.tile_pool(name="s_pool", bufs=2) as s_pool,
            tc.tile_pool(name="out_pool", bufs=2) as out_pool,
            tc.tile_pool(name="ps_pair", bufs=2, space="PSUM") as ps_pair,
            tc.tile_pool(name="ps_ef", bufs=2, space="PSUM") as ps_ef,
            tc.tile_pool(name="ps_ot", bufs=1, space="PSUM") as ps_ot,
            tc.tile_pool(name="ps_l", bufs=1, space="PSUM") as ps_l,
        ):
            # ---- persistent SBUF tensors ----
            x8_sb = pp.tile([P, NDC, S], FP8, name="x8_sb")
            xb_sb = pp.tile([P, NDC, S], BF16, name="xb_sb")
            wk8_sb = pp.tile([P, NDC, DS], FP8, name="wk8_sb")
            wv8_sb = pp.tile([P, NDC, DS], FP8, name="wv8_sb")
            wsb_sb = pp.tile([P, NDC, DS], BF16, name="wsb_sb")
            wout_sb = pp.tile([P, HPC, D], F32R, name="wout_sb")
            kt8_sb = pp.tile([P, HPC, S], FP8, name="kt8_sb")
            v8_sb = pp.tile([P, NKT, DS], FP8, name="v8_sb")
            ut_sb = pp.tile([P, HPC, S], F32R, name="ut_sb")
            ones8_sb = pp.tile([P, 2, P], FP8, name="ones8_sb")
            mcomb_sb = pp.tile([P, 2, 2 * P], BF16, name="mcomb_sb")
            m00c_sb = pp.tile([P, 2, 2 * P], BF16, name="m00c_sb")

            def pcopy(dst, src_):
                # psum->sbuf copies: split halves across ACT+DVE so the
                # PSUM bank frees in half the latency
                w = dst.shape[-1]
                h = w // 2
                nc.scalar.copy(dst[:, :h], src_[:, :h])
                nc.vector.tensor_copy(dst[:, h:], src_[:, h:])

            import contextlib

            loop_ctx = (
                tc.For_i(0, loop_n, 1) if loop_n else contextlib.nullcontext()
            )
            with loop_ctx:
              for _rep in range(repeats):
                # ---- weights + consts, one batched DMA each (the two HWDGE
                # rings have a large per-dma_start fixed cost) ----
                nc.sync.dma_start(
                    wk8_sb[:], wk8[:, :].rearrange("(c p) n -> p c n", p=P)
                )
                nc.sync.dma_start(
                    wv8_sb[:], wv8[:, :].rearrange("(c p) n -> p c n", p=P)
                )
                nc.sync.dma_start(ones8_sb[:], ones8[:])
                nc.sync.dma_start(mcomb_sb[:], mcomb[:])
                nc.sync.dma_start(m00c_sb[:], m00comb[:])
                nc.sync.dma_start(
                    wsb_sb[:], wsb[:, :].rearrange("(c p) n -> p c n", p=P)
                )
                nc.sync.dma_start(
                    wout_sb[:], woutT[:, :].rearrange("(c p) n -> p c n", p=P)
                )

                for j in range(NQC):
                    # ---- x chunk DMA (batched over the 8 D-chunks) ----
                    nc.sync.dma_start(
                        x8_sb[:, :, ts(j, QC)],
                        x8[:, ts(j, QC)].rearrange("(c p) q -> p c q", p=P),
                    )
                    nc.sync.dma_start(
                        xb_sb[:, :, ts(j, QC)],
                        xb[:, ts(j, QC)].rearrange("(c p) q -> p c q", p=P),
                    )

                    # ---- B_j: KT[h] chunk j  (fp8 DR, filler-bank pool) ----
                    for h in range(HPC if "B" in phases else 0):
                        ps = ps_ef.tile([P, QC], F32, name="ps_b", tag="ps_ef")
                        for cp in range(NDP):
                            nc.tensor.matmul(
                                ps[:],
                                wk8_sb[:, 2 * cp : 2 * cp + 2, ts(h, HD)],
                                x8_sb[:, 2 * cp : 2 * cp + 2, ts(j, QC)],
                                start=(cp == 0),
                                stop=(cp == NDP - 1),
                                perf_mode=DR,
                            )
                        pcopy(kt8_sb[:, h, ts(j, QC)], ps[:])

                    # ---- C_j: V tiles 4j..4j+3  (fp8 DR, filler-bank pool) ----
                    for st in range(*((4 * j, 4 * j + 4) if "C" in phases else (0, 0))):
                        ps = ps_ef.tile([P, QC], F32, name="ps_c", tag="ps_ef")
                        for cp in range(NDP):
                            nc.tensor.matmul(
                                ps[:, :DS],
                                x8_sb[:, 2 * cp : 2 * cp + 2, ts(st, P)],
                                wv8_sb[:, 2 * cp : 2 * cp + 2, :],
                                start=(cp == 0),
                                stop=(cp == NDP - 1),
                                perf_mode=DR,
                            )
                        pcopy(v8_sb[:, st, :], ps[:, :DS])

                    # ---- F (prev chunk) + E (this chunk): emitted as PE
                    # filler interleaved with D's units.  Both rotate through
                    # the shared 1-bank ps_ef pool so they never compete with
                    # the score tiles (ps_a) for PSUM. ----
                    def emit_F_qt(qt, ob4, slot):
                        for nch in range(2):
                            ps = ps_ef.tile([P, QC], F32, name="ps_f", tag="ps_ef")
                            for m in range(HPC):
                                nc.tensor.matmul(
                                    ps[:],
                                    ut_sb[:, m, ts(qt, P)],
                                    wout_sb[:, m, ts(nch, QC)],
                                    start=(m == 0),
                                    stop=(m == HPC - 1),
                                )
                            pcopy(ob4[:, slot, ts(nch, QC)], ps[:])

                    def emit_F_dma(jj, ob4):
                        nc.sync.dma_start(
                            part[ts(jj, QC), :].rearrange(
                                "(t p) d -> p t d", p=P
                            ),
                            ob4[:],
                        )
                        if jj == NQC - 1:
                            nc.sync.dma_start(tick[:, :], ob4[0:1, 3, 0:8])

                    # ---- D_j: attention, heads SEQUENTIAL (halves the live
                    # ot/lb banks), score pairs in 2-bank [P,2,QC] PSUM tiles
                    # (one exp instruction per pair), E/F matmuls injected as
                    # PE filler between pairs ----
                    npairs = 2 * j + 2

                    def emit_E_head(h):
                        ps = ps_ef.tile([P, QC], F32, name="ps_ej", tag="ps_ef")
                        for c in range(NDC):
                            nc.tensor.matmul(
                                ps[:],
                                wsb_sb[:, c, ts(h, HD)],
                                xb_sb[:, c, ts(j, QC)],
                                start=(c == 0),
                                stop=(c == NDC - 1),
                            )
                        pcopy(ut_sb[:, h, ts(j, QC)], ps[:])

                    # filler closures: F qt-groups for chunk j-1 (E heads are
                    # pinned inside their own head's pair loop below)
                    fillers = []
                    ob4 = None
                    if j > 0 and "F" in phases:
                        ob4 = out_pool.tile([P, 4, D], BF16, name="ob4", tag="ob")
                        for i, qt in enumerate(range(4 * (j - 1), 4 * j)):
                            fillers.append(
                                lambda qt=qt, i=i: emit_F_qt(qt, ob4, i)
                            )

                    total_units = 2 * npairs if "D" in phases else 0
                    inj = {}
                    if total_units and fillers:
                        for fi, f in enumerate(fillers):
                            pos = max(
                                1, (fi + 1) * total_units // (len(fillers) + 1)
                            )
                            inj.setdefault(pos, []).append(f)

                    uidx = 0
                    for h in range(HPC if "D" in phases else 0):
                        oth = ps_ot.tile([P, QC], F32, name="oth", tag="ps_ot")
                        lbh = ps_l.tile([P, QC], F32, name="lbh", tag="ps_l")

                        def emit_pvl(p, c00, pt2, h=h, oth=oth, lbh=lbh):
                            kt0 = 2 * p
                            nc.tensor.matmul(
                                oth[:, c00:],
                                v8_sb[:, kt0 : kt0 + 2, ts(h, HD)],
                                pt2[:, :, c00:],
                                start=(p == 0),
                                stop=(p == npairs - 1),
                                perf_mode=DR,
                            )
                            nc.tensor.matmul(
                                lbh[:, c00:],
                                ones8_sb[:],
                                pt2[:, :, c00:],
                                start=(p == 0),
                                stop=(p == npairs - 1),
                                perf_mode=DR,
                            )

                        pending = None
                        for p in range(npairs):
                            kt0 = 2 * p
                            c00 = max(0, P * kt0 - QC * j)
                            diag = kt0 >= 4 * j
                            ep2 = ps_pair.tile(
                                [P, 2, QC], F32, name="ep2", tag="pair"
                            )
                            pt2 = pt_pool.tile(
                                [P, 2, QC], FP8, name="pt2", tag="pt2"
                            )
                            for sub in range(2):
                                nc.tensor.matmul(
                                    ep2[:, sub, c00:],
                                    kt8_sb[:, h, ts(kt0 + sub, P)],
                                    kt8_sb[:, h, QC * j + c00 : QC * (j + 1)],
                                    start=True,
                                    stop=True,
                                )
                            if diag:
                                # one +100 additive causal mask op per diag
                                # pair, covering both subtiles' diag regions.
                                # (j=0,p=0) uses m00comb (-4 at [0,0]): q=0's
                                # single softmax term is scaled by e^4 in
                                # numerator and denominator (cancels) to keep
                                # its fp8 value away from underflow NaN.
                                mk = (
                                    m00c_sb
                                    if (j == 0 and p == 0)
                                    else mcomb_sb
                                )
                                nc.vector.scalar_tensor_tensor(
                                    ep2[:, :, c00 : c00 + 2 * P],
                                    ep2[:, :, c00 : c00 + 2 * P],
                                    1.0,
                                    mk[:],
                                    op0=MUL,
                                    op1=ADD,
                                )
                            nc.scalar.activation(
                                pt2[:, :, c00:],
                                ep2[:, :, c00:],
                                EXP,
                                scale=-1.0,
                            )
                            if pending is not None:
                                emit_pvl(*pending)
                            pending = (p, c00, pt2)
                            uidx += 1
                            for f in inj.get(uidx - 1, ()):
                                f()
                            if "E" in phases and p == npairs // 2:
                                emit_E_head(h)
                        if pending is not None:
                            emit_pvl(*pending)

                        # ---- head epilogue: ut += O/l ----
                        if j == 0:
                            # +eps on column q=0 only: its single softmax
                            # term can underflow fp8 to 0 -> 0*inf NaN
                            nc.vector.tensor_scalar_add(
                                lbh[:, 0:1], lbh[:, 0:1], 1e-30
                            )
                        li = rb_pool.tile([P, QC], F32, name="li", tag="li")
                        nc.vector.reciprocal(li[:], lbh[:])
                        sn = s_pool.tile([P, QC], F32, name="sn", tag="sn")
                        nc.vector.tensor_mul(sn[:], oth[:], li[:])
                        nc.vector.tensor_add(
                            ut_sb[:, h, ts(j, QC)],
                            ut_sb[:, h, ts(j, QC)].bitcast(F32),
                            sn[:],
                        )

                    if "D" not in phases:
                        for f in fillers:
                            f()
                        if "E" in phases:
                            emit_E_head(0)
                            emit_E_head(1)
                    if ob4 is not None:
                        emit_F_dma(j - 1, ob4)

                if "F" in phases:
                    ob4L = out_pool.tile([P, 4, D], BF16, name="ob4L", tag="ob")
                    for i, qt in enumerate(range(4 * (NQC - 1), 4 * NQC)):
                        emit_F_qt(qt, ob4L, i)
                    emit_F_dma(NQC - 1, ob4L)

    _legalize_waits(nc)
    return nc


_NC = None
D_MODE = "sym"  # compat knob for test.py; single implementation now


def _get_nc():
    global _NC
    if _NC is None:
        _NC = _build()
    return _NC


def build_in_maps(x, Wk, Wv, Wself, Wout):
    import ml_dtypes

    F8NP = ml_dtypes.float8_e4m3
    BFNP = ml_dtypes.bfloat16
    x = np.ascontiguousarray(np.asarray(x, dtype=np.float32))
    Wk = np.asarray(Wk, dtype=np.float32)
    Wv = np.asarray(Wv, dtype=np.float32)
    Wself = np.asarray(Wself, dtype=np.float32)
    Wout = np.asarray(Wout, dtype=np.float32)

    ks = np.float32(HD ** -0.25)
    ones8 = np.ones((P, 2 * P), F8NP)
    triu = np.triu(np.ones((P, P), np.float32))
    mtri = (1.0 - triu) * 100.0
    sub0 = np.concatenate([mtri, np.zeros((P, P), np.float32)], axis=1)
    sub1 = np.concatenate([np.full((P, P), 100.0, np.float32), mtri], axis=1)
    mcomb = np.concatenate([sub0, sub1], axis=1).astype(BFNP)
    m00f = mtri.copy()
    m00f[0, 0] = -4.0
    sub0b = np.concatenate([m00f, np.zeros((P, P), np.float32)], axis=1)
    m00comb = np.concatenate([sub0b, sub1], axis=1).astype(BFNP)

    in_maps = []
    for c in range(N_CORES):
        b, hp = divmod(c, 4)
        ds = slice(DS * hp, DS * (hp + 1))
        xT = np.ascontiguousarray(x[b].T)
        in_maps.append(
            {
                "x8": xT.astype(F8NP),
                "xb": xT.astype(BFNP),
                "wk8": np.ascontiguousarray((Wk[ds, :] * ks).T).astype(F8NP),
                "wv8": np.ascontiguousarray(Wv[ds, :].T).astype(F8NP),
                "wsb": np.ascontiguousarray(Wself[ds, :].T).astype(BFNP),
                "woutT": np.ascontiguousarray(Wout[:, ds].T),
                "ones8": ones8,
                "mcomb": mcomb,
                "m00comb": m00comb,
            }
        )
    return in_maps


def kernel(x, Wk, Wv, Wself, Wout):
    nc = _get_nc()
    in_maps = build_in_maps(x, Wk, Wv, Wself, Wout)
    res = run_bass_kernel_spmd(nc, in_maps, core_ids=list(range(N_CORES)))

    out = np.empty((B, S, D), np.float32)
    for b in range(B):
        acc = np.zeros((S, D), np.float32)
        for hp in range(4):
            acc += res.results[4 * b + hp]["part"].astype(np.float32)
        out[b] = acc
    return out


# revision 26
# speedup vs baseline: 2.0576x; 1.0619x over previous
"""Trainium2 Bass kernel for nn_EnergyFunction (dense transformer block).

Reference math (B=2, S=2048, D=1024, H=8 heads, hd=128):
    K  = x @ Wk.T            [B,S,D] -> heads [B,H,S,hd]
    V  = x @ Wv.T
    E  = (K K^T)/sqrt(hd)    per head, causal mask (q >= k allowed)
    P  = softmax(-E, axis=k)
    O  = P @ V               -> [B,S,D]
    out = (O + x @ Wself.T) @ Wout.T

Sharding (8 cores): core c -> batch b=c//4, head pair hp=c%4 (heads 2hp,2hp+1,
dims ds=[256*hp, 256*hp+256)).  Each core computes
    partial_c = (O_heads + x @ Wself.T[:,ds]) @ Wout.T[ds,:]   [S, D]
and the host sums the 4 partials per batch (row/column-parallel Wout split).

Design notes (vs the fp32r baseline, ~1.3x faster steady-state):
  * fp8e4m3 + DoubleRow matmuls (2 rows/cycle) for every contraction >= 256:
    K/V projections (x8/wk8/wv8 quantized on host), P@V and the softmax
    denominator (ones) matmuls (P stored fp8 pairs [P,2,QC]).  Scores stay
    fp8 non-DR (contraction = hd = 128, no DR win).  The Wself path is
    precision-critical (it dominates the output) and stays bf16 (fp8 there
    measures 4e-2 rel err); U@Wout stays fp32r.  End-to-end rel err 7.2e-3
    on hardware (gate 2e-2).
  * Causal mask applied as one +100 additive-mask DVE op per diagonal k-tile
    pair on the PSUM scores BEFORE exp, so exp writes fp8 directly.  Max
    unnormalized P on this data ~46 vs the 240 fp8 limit.  Tile (0,0) uses
    m00comb (-4 at [0,0]): q=0's single softmax term is scaled by e^4 in
    numerator and denominator (cancels) so its fp8 value cannot underflow to
    0 (which made row 0 go 0*inf=NaN); a j=0 epsilon on lb col 0 is a
    second guard.
  * Heads run SEQUENTIALLY within a q-chunk so only one ot/lb accumulator
    pair is live; score k-tile pairs go in 2-bank [P,2,QC] PSUM tiles with
    a single exp instruction per pair (halves ACT instruction overhead).
    PSUM budget exactly 8 banks: ps_pair 2x2 (scores), ps_ef 2 (B/C/E/F),
    ps_ot 1, ps_l 1.
  * B_j/C_j (next K/V chunks), E_j (self proj) and F_{j-1} (out proj) are
    emitted as PE filler between D's score pairs: the TensorE only reaches
    its full 2.4 GHz p-state when continuously busy, so it must never idle
    while ACT chews through the exps.
  * DMAs are batched into a handful of large multi-dim transfers (the two
    HWDGE rings have a big per-dma_start fixed cost; this took the
    B+C-only variant from 76us to 35us).  Output partials ship as bf16.
"""

import os
import sys

import numpy as np

if "/opt/trn_rl_repo" not in sys.path:
    sys.path.insert(0, "/opt/trn_rl_repo")

import concourse.bass as bass
import concourse.mybir as mybir
import concourse.tile as tile
from concourse.bass import ts
from concourse.bass_utils import run_bass_kernel_spmd

B, S, D = 2, 2048, 1024
H = 8
HD = D // H          # 128 head dim
HPC = 2              # heads per core
DS = HPC * HD        # 256 dims per core
N_CORES = 8
P = 128              # partitions
QC = 512             # q chunk width
NQC = S // QC        # 4 q chunks
NKT = S // P         # 16 k tiles
NDC = D // P         # 8 contraction chunks over D
NDP = NDC // 2       # 4 DoubleRow contraction pair-chunks

F32 = mybir.dt.float32
F32R = mybir.dt.float32r
BF16 = mybir.dt.bfloat16
FP8 = mybir.dt.float8e4
EXP = mybir.ActivationFunctionType.Exp
DR = mybir.MatmulPerfMode.DoubleRow
MUL = mybir.AluOpType.mult
ADD = mybir.AluOpType.add


def _legalize_waits(nc):
    """This toolchain's walrus rejects >1 semaphore wait on several
    instruction structs (Drain/CTRL allows none, Matmult/Ldweights S3_LW
    allows one).  Hoist excess waits onto same-engine NOPs placed
    immediately before the offending instruction."""
    for blk in nc.main_func.blocks:
        insts = blk.instructions
        new = []
        changed = False
        for ins in insts:
            si = ins.sync_info
            if si is not None and si.on_wait:
                allow = 0 if ins.opcode == "Drain" else 1
                waits = list(si.on_wait)
                if len(waits) > allow:
                    cut = len(waits) - allow
                    for k, w in enumerate(waits[:cut]):
                        nop = mybir.InstNoOp(
                            name=f"{ins.name}-wsplit{k}", engine=ins.engine
                        )
                        nop.sync_info = mybir.SyncInfo(on_wait=[w], on_update=[])
                        new.append(nop)
                    ins.sync_info = mybir.SyncInfo(
                        on_wait=waits[cut:], on_update=list(si.on_update)
                    )
                    changed = True
            new.append(ins)
        if changed:
            blk.instructions = new


def _build(repeats=1, loop_n=None, d_mode=None, phases="BCDEF", **_knobs):
    """loop_n: timing-only mode — wrap the body in a device-side For_i loop
    so NEFF execution time dominates the ~200 ms axon RPC floor."""
    nc = bass.Bass()

    x8 = nc.dram_tensor("x8", [D, S], FP8, kind="ExternalInput")
    xb = nc.dram_tensor("xb", [D, S], BF16, kind="ExternalInput")
    wk8 = nc.dram_tensor("wk8", [D, DS], FP8, kind="ExternalInput")
    wv8 = nc.dram_tensor("wv8", [D, DS], FP8, kind="ExternalInput")
    wsb = nc.dram_tensor("wsb", [D, DS], BF16, kind="ExternalInput")
    woutT = nc.dram_tensor("woutT", [DS, D], F32R, kind="ExternalInput")
    ones8 = nc.dram_tensor("ones8", [P, 2 * P], FP8, kind="ExternalInput")
    mcomb = nc.dram_tensor("mcomb", [P, 4 * P], BF16, kind="ExternalInput")
    m00comb = nc.dram_tensor("m00comb", [P, 4 * P], BF16, kind="ExternalInput")
    part = nc.dram_tensor("part", [S, D], BF16, kind="ExternalOutput")
    # tiny completion-marker output for timing harnesses
    tick = nc.dram_tensor("tick", [1, 8], BF16, kind="ExternalOutput")

    with tile.TileContext(nc) as tc:
        with (
            tc.tile_pool(name="persist", bufs=1) as pp,
            tc.tile_pool(name="pt_pool", bufs=8) as pt_pool,
            tc.tile_pool(name="rb_pool", bufs=2) as rb_pool,
            tc.tile_pool(name="s_pool", bufs=2) as s_pool,
            tc.tile_pool(name="out_pool", bufs=2) as out_pool,
            tc.tile_pool(name="ps_pair", bufs=2, space="PSUM") as ps_pair,
            tc.tile_pool(name="ps_ef", bufs=2, space="PSUM") as ps_ef,
            tc.tile_pool(name="ps_ot", bufs=1, space="PSUM") as ps_ot,
            tc.tile_pool(name="ps_l", bufs=1, space="PSUM") as ps_l,
        ):
            # ---- persistent SBUF tensors ----
            x8_sb = pp.tile([P, NDC, S], FP8, name="x8_sb")
            xb_sb = pp.tile([P, NDC, S], BF16, name="xb_sb")
            wk8_sb = pp.tile([P, NDC, DS], FP8, name="wk8_sb")
            wv8_sb = pp.tile([P, NDC, DS], FP8, name="wv8_sb")
            wsb_sb = pp.tile([P, NDC, DS], BF16, name="wsb_sb")
            wout_sb = pp.tile([P, HPC, D], F32R, name="wout_sb")
            kt8_sb = pp.tile([P, HPC, S], FP8, name="kt8_sb")
            v8_sb = pp.tile([P, NKT, DS], FP8, name="v8_sb")
            ut_sb = pp.tile([P, HPC, S], F32R, name="ut_sb")
            ones8_sb = pp.tile([P, 2, P], FP8, name="ones8_sb")
            mcomb_sb = pp.tile([P, 2, 2 * P], BF16, name="mcomb_sb")
            m00c_sb = pp.tile([P, 2, 2 * P], BF16, name="m00c_sb")

            def pcopy(dst, src_, act_frac=0.5):
                # psum->sbuf copies split across ACT+DVE; act_frac tunes how
                # much lands on ACT (exp throughput is the D-phase limiter,
                # so E/F copies push most of their width onto DVE)
                w = dst.shape[-1]
                h = (int(w * act_frac) // 64) * 64
                if h == 0:
                    nc.vector.tensor_copy(dst[:], src_[:])
                    return
                nc.scalar.copy(dst[:, :h], src_[:, :h])
                nc.vector.tensor_copy(dst[:, h:], src_[:, h:])

            import contextlib

            loop_ctx = (
                tc.For_i(0, loop_n, 1) if loop_n else contextlib.nullcontext()
            )
            with loop_ctx:
              for _rep in range(repeats):
                # ---- weights + consts, one batched DMA each (the two HWDGE
                # rings have a large per-dma_start fixed cost) ----
                nc.sync.dma_start(
                    wk8_sb[:], wk8[:, :].rearrange("(c p) n -> p c n", p=P)
                )
                nc.sync.dma_start(
                    x8_sb[:, :, ts(0, QC)],
                    x8[:, ts(0, QC)].rearrange("(c p) q -> p c q", p=P),
                )
                nc.sync.dma_start(
                    wv8_sb[:], wv8[:, :].rearrange("(c p) n -> p c n", p=P)
                )
                nc.sync.dma_start(
                    xb_sb[:, :, ts(0, QC)],
                    xb[:, ts(0, QC)].rearrange("(c p) q -> p c q", p=P),
                )
                nc.sync.dma_start(ones8_sb[:], ones8[:])
                nc.sync.dma_start(mcomb_sb[:], mcomb[:])
                nc.sync.dma_start(m00c_sb[:], m00comb[:])
                nc.sync.dma_start(
                    wsb_sb[:], wsb[:, :].rearrange("(c p) n -> p c n", p=P)
                )
                nc.sync.dma_start(
                    wout_sb[:], woutT[:, :].rearrange("(c p) n -> p c n", p=P)
                )

                for j in range(NQC):
                    # ---- x chunk DMA (chunk 0 is prefetched above) ----
                    if j > 0:
                        nc.sync.dma_start(
                            x8_sb[:, :, ts(j, QC)],
                            x8[:, ts(j, QC)].rearrange("(c p) q -> p c q", p=P),
                        )
                        nc.sync.dma_start(
                            xb_sb[:, :, ts(j, QC)],
                            xb[:, ts(j, QC)].rearrange("(c p) q -> p c q", p=P),
                        )

                    # ---- B_j: KT[h] chunk j  (fp8 DR, filler-bank pool) ----
                    for h in range(HPC if "B" in phases else 0):
                        ps = ps_ef.tile([P, QC], F32, name="ps_b", tag="ps_ef")
                        for cp in range(NDP):
                            nc.tensor.matmul(
                                ps[:],
                                wk8_sb[:, 2 * cp : 2 * cp + 2, ts(h, HD)],
                                x8_sb[:, 2 * cp : 2 * cp + 2, ts(j, QC)],
                                start=(cp == 0),
                                stop=(cp == NDP - 1),
                                perf_mode=DR,
                            )
                        pcopy(kt8_sb[:, h, ts(j, QC)], ps[:])

                    # ---- C_j: V tiles 4j..4j+3  (fp8 DR, filler-bank pool) ----
                    for st in range(*((4 * j, 4 * j + 4) if "C" in phases else (0, 0))):
                        ps = ps_ef.tile([P, QC], F32, name="ps_c", tag="ps_ef")
                        for cp in range(NDP):
                            nc.tensor.matmul(
                                ps[:, :DS],
                                x8_sb[:, 2 * cp : 2 * cp + 2, ts(st, P)],
                                wv8_sb[:, 2 * cp : 2 * cp + 2, :],
                                start=(cp == 0),
                                stop=(cp == NDP - 1),
                                perf_mode=DR,
                            )
                        pcopy(v8_sb[:, st, :], ps[:, :DS])

                    # ---- F (prev chunk) + E (this chunk): emitted as PE
                    # filler interleaved with D's units.  Both rotate through
                    # the shared 1-bank ps_ef pool so they never compete with
                    # the score tiles (ps_a) for PSUM. ----
                    def emit_F_qt(qt, ob4, slot):
                        for nch in range(2):
                            ps = ps_ef.tile([P, QC], F32, name="ps_f", tag="ps_ef")
                            for m in range(HPC):
                                nc.tensor.matmul(
                                    ps[:],
                                    ut_sb[:, m, ts(qt, P)],
                                    wout_sb[:, m, ts(nch, QC)],
                                    start=(m == 0),
                                    stop=(m == HPC - 1),
                                )
                            pcopy(ob4[:, slot, ts(nch, QC)], ps[:], act_frac=0.25)

                    def emit_F_dma(jj, ob4):
                        nc.sync.dma_start(
                            part[ts(jj, QC), :].rearrange(
                                "(t p) d -> p t d", p=P
                            ),
                            ob4[:],
                        )
                        if jj == NQC - 1:
                            nc.sync.dma_start(tick[:, :], ob4[0:1, 3, 0:8])

                    # ---- D_j: attention, heads SEQUENTIAL (halves the live
                    # ot/lb banks), score pairs in 2-bank [P,2,QC] PSUM tiles
                    # (one exp instruction per pair), E/F matmuls injected as
                    # PE filler between pairs ----
                    npairs = 2 * j + 2

                    def emit_E_head(h):
                        ps = ps_ef.tile([P, QC], F32, name="ps_ej", tag="ps_ef")
                        for c in range(NDC):
                            nc.tensor.matmul(
                                ps[:],
                                wsb_sb[:, c, ts(h, HD)],
                                xb_sb[:, c, ts(j, QC)],
                                start=(c == 0),
                                stop=(c == NDC - 1),
                            )
                        pcopy(ut_sb[:, h, ts(j, QC)], ps[:], act_frac=0.25)

                    # filler closures: F qt-groups for chunk j-1 (E heads are
                    # pinned inside their own head's pair loop below)
                    fillers = []
                    ob4 = None
                    if j > 0 and "F" in phases:
                        ob4 = out_pool.tile([P, 4, D], BF16, name="ob4", tag="ob")
                        for i, qt in enumerate(range(4 * (j - 1), 4 * j)):
                            fillers.append(
                                lambda qt=qt, i=i: emit_F_qt(qt, ob4, i)
                            )

                    total_units = 2 * npairs if "D" in phases else 0
                    inj = {}
                    if total_units and fillers:
                        for fi, f in enumerate(fillers):
                            pos = max(
                                1, (fi + 1) * total_units // (len(fillers) + 1)
                            )
                            inj.setdefault(pos, []).append(f)

                    uidx = 0
                    for h in range(HPC if "D" in phases else 0):
                        oth = ps_ot.tile([P, QC], F32, name="oth", tag="ps_ot")
                        lbh = ps_l.tile([P, QC], F32, name="lbh", tag="ps_l")

                        def emit_pvl(p, c00, pt2, h=h, oth=oth, lbh=lbh):
                            kt0 = 2 * p
                            nc.tensor.matmul(
                                oth[:, c00:],
                                v8_sb[:, kt0 : kt0 + 2, ts(h, HD)],
                                pt2[:, :, c00:],
                                start=(p == 0),
                                stop=(p == npairs - 1),
                                perf_mode=DR,
                            )
                            nc.tensor.matmul(
                                lbh[:, c00:],
                                ones8_sb[:],
                                pt2[:, :, c00:],
                                start=(p == 0),
                                stop=(p == npairs - 1),
                                perf_mode=DR,
                            )

                        pending = None
                        for p in range(npairs):
                            kt0 = 2 * p
                            c00 = max(0, P * kt0 - QC * j)
                            diag = kt0 >= 4 * j
                            ep2 = ps_pair.tile(
                                [P, 2, QC], F32, name="ep2", tag="pair"
                            )
                            pt2 = pt_pool.tile(
                                [P, 2, QC], FP8, name="pt2", tag="pt2"
                            )
                            for sub in range(2):
                                nc.tensor.matmul(
                                    ep2[:, sub, c00:],
                                    kt8_sb[:, h, ts(kt0 + sub, P)],
                                    kt8_sb[:, h, QC * j + c00 : QC * (j + 1)],
                                    start=True,
                                    stop=True,
                                )
                            if diag:
                                # one +100 additive causal mask op per diag
                                # pair, covering both subtiles' diag regions.
                                # (j=0,p=0) uses m00comb (-4 at [0,0]): q=0's
                                # single softmax term is scaled by e^4 in
                                # numerator and denominator (cancels) to keep
                                # its fp8 value away from underflow NaN.
                                mk = (
                                    m00c_sb
                                    if (j == 0 and p == 0)
                                    else mcomb_sb
                                )
                                nc.vector.scalar_tensor_tensor(
                                    ep2[:, :, c00 : c00 + 2 * P],
                                    ep2[:, :, c00 : c00 + 2 * P],
                                    1.0,
                                    mk[:],
                                    op0=MUL,
                                    op1=ADD,
                                )
                            nc.scalar.activation(
                                pt2[:, :, c00:],
                                ep2[:, :, c00:],
                                EXP,
                                scale=-1.0,
                            )
                            if pending is not None:
                                emit_pvl(*pending)
                            pending = (p, c00, pt2)
                            uidx += 1
                            for f in inj.get(uidx - 1, ()):
                                f()
                            if "E" in phases and p == npairs // 2:
                                emit_E_head(h)
                        if pending is not None:
                            emit_pvl(*pending)

                        # ---- head epilogue: ut += O/l ----
                        if j == 0:
                            # +eps on column q=0 only: its single softmax
                            # term can underflow fp8 to 0 -> 0*inf NaN
                            nc.vector.tensor_scalar_add(
                                lbh[:, 0:1], lbh[:, 0:1], 1e-30
                            )
                        li = rb_pool.tile([P, QC], F32, name="li", tag="li")
                        nc.vector.reciprocal(li[:], lbh[:])
                        sn = s_pool.tile([P, QC], F32, name="sn", tag="sn")
                        nc.vector.tensor_mul(sn[:], oth[:], li[:])
                        nc.vector.tensor_add(
                            ut_sb[:, h, ts(j, QC)],
                            ut_sb[:, h, ts(j, QC)].bitcast(F32),
                            sn[:],
                        )

                    if "D" not in phases:
                        for f in fillers:
                            f()
                        if "E" in phases:
                            emit_E_head(0)
                            emit_E_head(1)
                    if ob4 is not None:
                        emit_F_dma(j - 1, ob4)

                if "F" in phases:
                    ob4L = out_pool.tile([P, 4, D], BF16, name="ob4L", tag="ob")
                    for i, qt in enumerate(range(4 * (NQC - 1), 4 * NQC)):
                        emit_F_qt(qt, ob4L, i)
                    emit_F_dma(NQC - 1, ob4L)

    _legalize_waits(nc)
    return nc


_NC = None
D_MODE = "sym"  # compat knob for test.py; single implementation now


def _get_nc():
    global _NC
    if _NC is None:
        _NC = _build()
    return _NC


def build_in_maps(x, Wk, Wv, Wself, Wout):
    import ml_dtypes

    F8NP = ml_dtypes.float8_e4m3
    BFNP = ml_dtypes.bfloat16
    x = np.ascontiguousarray(np.asarray(x, dtype=np.float32))
    Wk = np.asarray(Wk, dtype=np.float32)
    Wv = np.asarray(Wv, dtype=np.float32)
    Wself = np.asarray(Wself, dtype=np.float32)
    Wout = np.asarray(Wout, dtype=np.float32)

    ks = np.float32(HD ** -0.25)
    ones8 = np.ones((P, 2 * P), F8NP)
    triu = np.triu(np.ones((P, P), np.float32))
    mtri = (1.0 - triu) * 100.0
    sub0 = np.concatenate([mtri, np.zeros((P, P), np.float32)], axis=1)
    sub1 = np.concatenate([np.full((P, P), 100.0, np.float32), mtri], axis=1)
    mcomb = np.concatenate([sub0, sub1], axis=1).astype(BFNP)
    m00f = mtri.copy()
    m00f[0, 0] = -4.0
    sub0b = np.concatenate([m00f, np.zeros((P, P), np.float32)], axis=1)
    m00comb = np.concatenate([sub0b, sub1], axis=1).astype(BFNP)

    in_maps = []
    for c in range(N_CORES):
        b, hp = divmod(c, 4)
        ds = slice(DS * hp, DS * (hp + 1))
        xT = np.ascontiguousarray(x[b].T)
        in_maps.append(
            {
                "x8": xT.astype(F8NP),
                "xb": xT.astype(BFNP),
                "wk8": np.ascontiguousarray((Wk[ds, :] * ks).T).astype(F8NP),
                "wv8": np.ascontiguousarray(Wv[ds, :].T).astype(F8NP),
                "wsb": np.ascontiguousarray(Wself[ds, :].T).astype(BFNP),
                "woutT": np.ascontiguousarray(Wout[:, ds].T),
                "ones8": ones8,
                "mcomb": mcomb,
                "m00comb": m00comb,
            }
        )
    return in_maps


def kernel(x, Wk, Wv, Wself, Wout):
    nc = _get_nc()
    in_maps = build_in_maps(x, Wk, Wv, Wself, Wout)
    res = run_bass_kernel_spmd(nc, in_maps, core_ids=list(range(N_CORES)))

    out = np.empty((B, S, D), np.float32)
    for b in range(B):
        acc = np.zeros((S, D), np.float32)
        for hp in range(4):
            acc += res.results[4 * b + hp]["part"].astype(np.float32)
        out[b] = acc
    return out
